# revision 21
# baseline (speedup 1.0000x reference)
"""Segment-mean (MeanToERA5) Trainium2 kernel.

Computes per-cluster means of a [32, 8, 512, 512] fp32 tensor over the
flattened 512x512 spatial axis, for 4096 clusters given by `mapping`
([262144] int), matching jax.ops.segment_sum(flat.T, mapping)/counts.

Strategy (8 NeuronCores, SPMD):
  - Host: stable-argsort `mapping`; group the 4096 clusters into groups of
    G=32 consecutive clusters; lay out the data cluster-sorted and
    transposed as rows of [256 batch] fp32, padded per-group to a uniform
    row count 128*cpg so the program structure is identical on every
    core. Each core owns 512 clusters = 16 groups. Inputs are packed
    partition-major on the host so all DMAs are fully contiguous.
  - Device: build the per-chunk [128, 32] one-hot weights on DVE from
    compact (column-id, 1/count) vectors; per 128-row chunk one fp32
    matmul: stationary = one-hot, moving = data chunk [128, 256]. PSUM
    accumulates [512 clusters, 256 batch] c-major in 4 [128, 256] tiles;
    copy + DMA out at the end.
  - Host: assemble [4096, 256], transpose to [256, 4096] (the unshard).
"""

import sys
import time

if "/opt/trn_rl_repo" not in sys.path:
    sys.path.insert(0, "/opt/trn_rl_repo")

import numpy as np

import concourse.bacc as bacc
import concourse.tile as tile
from concourse import mybir
from concourse.bass_utils import run_bass_kernel_spmd

N_CLUSTERS = 4096
N = 512 * 512
B = 256
NCORES = 8
G = 32                      # clusters per group (= one-hot width)
GROUPS_PER_CORE = (N_CLUSTERS // NCORES) // G   # 16
CLUSTERS_PER_CORE = N_CLUSTERS // NCORES        # 512
NQ = CLUSTERS_PER_CORE // 128                   # psum tiles (4)

_program_cache = {}
LAST_EXEC_NS = None


def _build_program(cpg: int, loop: int = 1):
    """Build the SPMD bass program for `cpg` 128-row chunks per group.

    loop > 1 repeats the whole pipeline on-device (for benchmarking: one
    dispatch, `loop` executions)."""
    key = (cpg, loop)
    if key in _program_cache:
        return _program_cache[key]

    nchunks = GROUPS_PER_CORE * cpg    # chunks per core
    gpq = 128 // G                     # groups per psum tile (4)

    nc = bacc.Bacc("TRN2", target_bir_lowering=False, debug=False,
                   num_devices=NCORES)
    # x packed as [groups, 128 partitions, cpg*B] (host pre-permuted)
    x = nc.dram_tensor("x", [GROUPS_PER_CORE, 128, cpg * B],
                       mybir.dt.float32, kind="ExternalInput")
    # per-row one-hot column id and value, packed [128, nchunks]
    cid = nc.dram_tensor("cid", [128, nchunks], mybir.dt.float32,
                         kind="ExternalInput")
    val = nc.dram_tensor("val", [128, nchunks], mybir.dt.float32,
                         kind="ExternalInput")
    iota = nc.dram_tensor("iota", [128, G], mybir.dt.float32,
                          kind="ExternalInput")
    # output c-major: [512 clusters, 256 batch]
    out = nc.dram_tensor("out", [CLUSTERS_PER_CORE, B], mybir.dt.float32,
                         kind="ExternalOutput")

    xv, outv = x.ap(), out.ap()

    with tile.TileContext(nc) as tc:
        with (
            tc.tile_pool(name="xp", bufs=6) as xp,
            tc.tile_pool(name="ohp", bufs=1) as ohp,
            tc.tile_pool(name="ps", bufs=1, space="PSUM") as ps,
            tc.tile_pool(name="res", bufs=2) as resp,
        ):
            def body(_i=None):
                cidt = ohp.tile([128, nchunks], mybir.dt.float32,
                                name="cidt", tag="cidt")
                nc.scalar.dma_start(cidt[:], cid.ap())
                valt = ohp.tile([128, nchunks], mybir.dt.float32,
                                name="valt", tag="valt")
                nc.scalar.dma_start(valt[:], val.ap())
                iot = ohp.tile([128, G], mybir.dt.float32,
                               name="iot", tag="iot")
                nc.scalar.dma_start(iot[:], iota.ap())
                # expand to one-hot weights [128, nchunks, G] (per group,
                # so matmuls can start as soon as the first slice is ready)
                ohx = ohp.tile([128, nchunks, G], mybir.dt.float32,
                               name="ohx", tag="ohx")
                for g in range(GROUPS_PER_CORE):
                    s = slice(g * cpg, (g + 1) * cpg)
                    nc.vector.tensor_tensor(
                        out=ohx[:, s, :],
                        in0=cidt[:, s].unsqueeze(2)
                            .broadcast_to([128, cpg, G]),
                        in1=iot[:].unsqueeze(1).broadcast_to([128, cpg, G]),
                        op=mybir.AluOpType.is_equal,
                    )
                    nc.vector.tensor_tensor(
                        out=ohx[:, s, :],
                        in0=ohx[:, s, :],
                        in1=valt[:, s].unsqueeze(2)
                            .broadcast_to([128, cpg, G]),
                        op=mybir.AluOpType.mult,
                    )
                psum = [
                    ps.tile([128, B], mybir.dt.float32,
                            name=f"psum{q}", tag=f"psum{q}")
                    for q in range(NQ)
                ]
                for g in range(GROUPS_PER_CORE):
                    q, gq = divmod(g, gpq)
                    po = gq * G        # partition offset within psum tile
                    xt = xp.tile([128, cpg * B], mybir.dt.float32, tag="xt")
                    nc.sync.dma_start(xt[:], xv[g])
                    for t in range(cpg):
                        j = g * cpg + t
                        nc.tensor.matmul(
                            out=psum[q][po:po + G, :],
                            lhsT=ohx[:, j, :],
                            rhs=xt[:, t * B:(t + 1) * B],
                            start=(t == 0),
                            stop=(t == cpg - 1),
                            tile_position=(0, po),
                        )
                    if gq == gpq - 1:
                        # psum tile q complete: evacuate + write out now,
                        # overlapped with the remaining groups' DMA/matmuls
                        res = resp.tile([128, B], mybir.dt.float32,
                                        name=f"res{q}", tag="res")
                        nc.vector.tensor_copy(res[:], psum[q][:])
                        nc.scalar.dma_start(outv[q * 128:(q + 1) * 128, :],
                                            res[:])

            if loop == 1:
                body()
            else:
                with tc.For_i(0, loop, 1) as i:
                    body(i)

    nc.compile()
    _program_cache[key] = nc
    return nc


def _solve_bins(counts: np.ndarray):
    """Partition the 4096 clusters into 128 bins of exactly 32 clusters,
    equalizing bin row-sums (ideally all == 2048 -> zero padding). Returns
    (bin_of, slot_of) int arrays."""
    n_bins = N_CLUSTERS // G
    target = int(counts.sum()) // n_bins
    rng = np.random.default_rng(0)
    orderd = np.argsort(-counts)
    bins = [[] for _ in range(n_bins)]
    sums = np.zeros(n_bins, dtype=np.int64)
    nitems = np.zeros(n_bins, dtype=np.int64)
    for c in orderd:
        cand = np.where(nitems < G)[0]
        b = int(cand[np.argmin(sums[cand])])
        bins[b].append(int(c))
        sums[b] += counts[c]
        nitems[b] += 1
    for _ in range(300000):
        dev = sums - target
        over = np.where(dev > 0)[0]
        under = np.where(dev < 0)[0]
        if len(over) == 0 or len(under) == 0:
            break
        A = int(rng.choice(over))
        Bb = int(rng.choice(under))
        ca, cb = bins[A], bins[Bb]
        diff = counts[ca][:, None] - counts[cb][None, :]
        tot = np.abs(dev[A] - diff) + np.abs(dev[Bb] + diff)
        i, j = np.unravel_index(int(np.argmin(tot)), tot.shape)
        if tot[i, j] < abs(dev[A]) + abs(dev[Bb]):
            a, b2 = ca[i], cb[j]
            ca.remove(a), cb.remove(b2)
            ca.append(b2), cb.append(a)
            d = counts[a] - counts[b2]
            sums[A] -= d
            sums[Bb] += d
    bin_of = np.zeros(N_CLUSTERS, dtype=np.int64)
    slot_of = np.zeros(N_CLUSTERS, dtype=np.int64)
    for b, cl in enumerate(bins):
        bin_of[cl] = b
        slot_of[cl] = np.arange(len(cl))
    return bin_of, slot_of, int(sums.max())


def _prepare(output: np.ndarray, mapping: np.ndarray):
    """Host prep: returns (nc, in_maps, cpg, unperm)."""
    t0 = time.time()
    assert output.shape == (32, 8, 512, 512) and output.dtype == np.float32
    mapping = np.asarray(mapping).astype(np.int64).ravel()
    assert mapping.shape == (N,)

    data2d = output.reshape(B, N)
    counts = np.bincount(mapping, minlength=N_CLUSTERS).astype(np.int64)
    recip = (1.0 / np.maximum(counts, 1)).astype(np.float32)

    order = np.argsort(mapping, kind="stable")
    cum = np.zeros(N_CLUSTERS + 1, dtype=np.int64)
    np.cumsum(counts, out=cum[1:])

    n_groups = N_CLUSTERS // G
    # Bin-pack clusters into groups to minimize padding; fall back to
    # consecutive grouping if the packer leaves an oversized bin.
    bin_of, slot_of, maxsum = _solve_bins(counts)
    naive_max = int(np.add.reduceat(counts, np.arange(0, N_CLUSTERS, G)).max())
    if maxsum > naive_max:
        bin_of = np.arange(N_CLUSTERS) // G
        slot_of = np.arange(N_CLUSTERS) % G
        maxsum = naive_max
    cpg = max(1, int(np.ceil(maxsum / 128)))
    L = 128 * cpg

    # clusters in destination order (bin-major, slot order)
    dest_order = np.lexsort((slot_of, bin_of))
    glen = np.zeros(n_groups, dtype=np.int64)
    np.add.at(glen, bin_of, counts)
    rows_sorted = np.concatenate(
        [order[cum[c]:cum[c + 1]] for c in dest_order])
    gstart = np.zeros(n_groups + 1, dtype=np.int64)
    np.cumsum(glen, out=gstart[1:])

    # Padded row-id table [n_groups, L]; -1 = padding.
    pad_rows = np.full((n_groups, L), -1, dtype=np.int64)
    col = np.arange(L)
    valid = col[None, :] < glen[:, None]
    flat_src = np.zeros((n_groups, L), dtype=np.int64)
    flat_src[valid] = rows_sorted[
        (gstart[:-1][:, None] + np.minimum(col[None, :], glen[:, None] - 1))[valid]
    ]
    pad_rows[valid] = flat_src[valid]
    pad_rows = pad_rows.reshape(-1)        # [n_groups * L]
    vmask = pad_rows >= 0

    # Gather data rows (transposed): x_all[r] = data2d[:, pad_rows[r]]
    dataT = np.ascontiguousarray(data2d.T)          # [N, B]
    x_all = np.zeros((n_groups * L, B), dtype=np.float32)
    x_all[vmask] = dataT[pad_rows[vmask]]
    # pack partition-major: [g, t, p, b] -> [g, p, t*B + b]
    x_all = np.ascontiguousarray(
        x_all.reshape(n_groups, cpg, 128, B).transpose(0, 2, 1, 3)
    ).reshape(n_groups, 128, cpg * B)

    # Compact one-hot: per-row within-group column id and value 1/count.
    cid_all = np.zeros(n_groups * L, dtype=np.float32)
    val_all = np.zeros(n_groups * L, dtype=np.float32)
    clus = mapping[pad_rows[vmask]]
    cid_all[vmask] = slot_of[clus].astype(np.float32)
    val_all[vmask] = recip[clus]
    # where cluster c ended up in the concatenated [4096, B] device output
    unperm = bin_of * G + slot_of
    # pack [rows] -> [core][p][chunk]
    nchunks = GROUPS_PER_CORE * cpg

    def pack(a):
        return np.ascontiguousarray(
            a.reshape(NCORES, nchunks, 128).transpose(0, 2, 1))

    cid_all = pack(cid_all)
    val_all = pack(val_all)
    iota_np = np.broadcast_to(np.arange(G, dtype=np.float32), (128, G)).copy()

    t1 = time.time()
    nc = _build_program(cpg)

    in_maps = []
    for k in range(NCORES):
        in_maps.append({
            "x": x_all[k * GROUPS_PER_CORE:(k + 1) * GROUPS_PER_CORE],
            "cid": cid_all[k],
            "val": val_all[k],
            "iota": iota_np,
        })
    print(f"[kernel] host prep {t1 - t0:.2f}s  build+compile "
          f"{time.time() - t1:.2f}s  (cpg={cpg})", file=sys.stderr, flush=True)
    return nc, in_maps, cpg, unperm


def kernel(output: np.ndarray, mapping: np.ndarray) -> np.ndarray:
    nc, in_maps, _, unperm = _prepare(output, mapping)
    t2 = time.time()
    res = run_bass_kernel_spmd(nc, in_maps, list(range(NCORES)))
    t3 = time.time()
    full = np.concatenate([res.results[k]["out"] for k in range(NCORES)],
                          axis=0)                   # [4096, 256] device order
    full = full[unperm]                             # -> cluster order
    out = np.ascontiguousarray(full.T).reshape(32, 8, N_CLUSTERS)
    print(f"[kernel] run {t3 - t2:.2f}s", file=sys.stderr, flush=True)
    return out


# revision 22
# speedup vs baseline: 1.1501x; 1.1501x over previous
"""Segment-mean (MeanToERA5) Trainium2 kernel.

Computes per-cluster means of a [32, 8, 512, 512] fp32 tensor over the
flattened 512x512 spatial axis, for 4096 clusters given by `mapping`
([262144] int), matching jax.ops.segment_sum(flat.T, mapping)/counts.

Strategy (8 NeuronCores, SPMD):
  - Host: stable-argsort `mapping`; group the 4096 clusters into groups of
    G=32 consecutive clusters; lay out the data cluster-sorted and
    transposed as rows of [256 batch] fp32, padded per-group to a uniform
    row count 128*cpg so the program structure is identical on every
    core. Each core owns 512 clusters = 16 groups. Inputs are packed
    partition-major on the host so all DMAs are fully contiguous.
  - Device: build the per-chunk [128, 32] one-hot weights on DVE from
    compact (column-id, 1/count) vectors; per 128-row chunk one fp32
    matmul: stationary = one-hot, moving = data chunk [128, 256]. PSUM
    accumulates [512 clusters, 256 batch] c-major in 4 [128, 256] tiles;
    copy + DMA out at the end.
  - Host: assemble [4096, 256], transpose to [256, 4096] (the unshard).
"""

import sys
import time

if "/opt/trn_rl_repo" not in sys.path:
    sys.path.insert(0, "/opt/trn_rl_repo")

import numpy as np

import concourse.bacc as bacc
import concourse.tile as tile
from concourse import mybir
from concourse.bass_utils import run_bass_kernel_spmd

N_CLUSTERS = 4096
N = 512 * 512
B = 256
NCORES = 8
G = 32                      # clusters per group (= one-hot width)
GROUPS_PER_CORE = (N_CLUSTERS // NCORES) // G   # 16
CLUSTERS_PER_CORE = N_CLUSTERS // NCORES        # 512
NQ = CLUSTERS_PER_CORE // 128                   # psum tiles (4)

_program_cache = {}
LAST_EXEC_NS = None


def _build_program(cpg: int, loop: int = 1):
    """Build the SPMD bass program for `cpg` 128-row chunks per group.

    loop > 1 repeats the whole pipeline on-device (for benchmarking: one
    dispatch, `loop` executions)."""
    key = (cpg, loop)
    if key in _program_cache:
        return _program_cache[key]

    nchunks = GROUPS_PER_CORE * cpg    # chunks per core
    gpq = 128 // G                     # groups per psum tile (4)

    nc = bacc.Bacc("TRN2", target_bir_lowering=False, debug=False,
                   num_devices=NCORES)
    # x packed as [groups, 128 partitions, cpg*B] (host pre-permuted)
    x = nc.dram_tensor("x", [GROUPS_PER_CORE, 128, cpg * B],
                       mybir.dt.float32, kind="ExternalInput")
    # per-row one-hot column id and value, packed [128, nchunks]
    cid = nc.dram_tensor("cid", [128, nchunks], mybir.dt.float32,
                         kind="ExternalInput")
    val = nc.dram_tensor("val", [128, nchunks], mybir.dt.float32,
                         kind="ExternalInput")
    iota = nc.dram_tensor("iota", [128, G], mybir.dt.float32,
                          kind="ExternalInput")
    # output c-major: [512 clusters, 256 batch]
    out = nc.dram_tensor("out", [CLUSTERS_PER_CORE, B], mybir.dt.float32,
                         kind="ExternalOutput")

    xv, outv = x.ap(), out.ap()

    with tile.TileContext(nc) as tc:
        with (
            tc.tile_pool(name="xp", bufs=6) as xp,
            tc.tile_pool(name="ohp", bufs=1) as ohp,
            tc.tile_pool(name="ps", bufs=1, space="PSUM") as ps,
            tc.tile_pool(name="res", bufs=2) as resp,
        ):
            def body(_i=None):
                cidt = ohp.tile([128, nchunks], mybir.dt.float32,
                                name="cidt", tag="cidt")
                nc.sync.dma_start(cidt[:], cid.ap())
                valt = ohp.tile([128, nchunks], mybir.dt.float32,
                                name="valt", tag="valt")
                nc.sync.dma_start(valt[:], val.ap())
                iot = ohp.tile([128, G], mybir.dt.float32,
                               name="iot", tag="iot")
                nc.sync.dma_start(iot[:], iota.ap())
                # expand to one-hot weights [128, nchunks, G] (per group,
                # so matmuls can start as soon as the first slice is ready)
                ohx = ohp.tile([128, nchunks, G], mybir.dt.float32,
                               name="ohx", tag="ohx")
                for g in range(GROUPS_PER_CORE):
                    s = slice(g * cpg, (g + 1) * cpg)
                    nc.vector.tensor_tensor(
                        out=ohx[:, s, :],
                        in0=cidt[:, s].unsqueeze(2)
                            .broadcast_to([128, cpg, G]),
                        in1=iot[:].unsqueeze(1).broadcast_to([128, cpg, G]),
                        op=mybir.AluOpType.is_equal,
                    )
                    nc.vector.tensor_tensor(
                        out=ohx[:, s, :],
                        in0=ohx[:, s, :],
                        in1=valt[:, s].unsqueeze(2)
                            .broadcast_to([128, cpg, G]),
                        op=mybir.AluOpType.mult,
                    )
                psum = [
                    ps.tile([128, B], mybir.dt.float32,
                            name=f"psum{q}", tag=f"psum{q}")
                    for q in range(NQ)
                ]
                for g in range(GROUPS_PER_CORE):
                    q, gq = divmod(g, gpq)
                    po = gq * G        # partition offset within psum tile
                    xt = xp.tile([128, cpg * B], mybir.dt.float32, tag="xt")
                    nc.sync.dma_start(xt[:], xv[g])
                    for t in range(cpg):
                        j = g * cpg + t
                        nc.tensor.matmul(
                            out=psum[q][po:po + G, :],
                            lhsT=ohx[:, j, :],
                            rhs=xt[:, t * B:(t + 1) * B],
                            start=(t == 0),
                            stop=(t == cpg - 1),
                            tile_position=(0, po),
                        )
                    if gq == gpq - 1:
                        # psum tile q complete: evacuate + write out now,
                        # overlapped with the remaining groups' DMA/matmuls
                        res = resp.tile([128, B], mybir.dt.float32,
                                        name=f"res{q}", tag="res")
                        nc.vector.tensor_copy(res[:], psum[q][:])
                        nc.sync.dma_start(outv[q * 128:(q + 1) * 128, :],
                                          res[:])

            if loop == 1:
                body()
            else:
                with tc.For_i(0, loop, 1) as i:
                    body(i)

    nc.compile()
    _program_cache[key] = nc
    return nc


def _solve_bins(counts: np.ndarray):
    """Partition the 4096 clusters into 128 bins of exactly 32 clusters,
    equalizing bin row-sums (ideally all == 2048 -> zero padding). Returns
    (bin_of, slot_of) int arrays."""
    n_bins = N_CLUSTERS // G
    target = int(counts.sum()) // n_bins
    rng = np.random.default_rng(0)
    orderd = np.argsort(-counts)
    bins = [[] for _ in range(n_bins)]
    sums = np.zeros(n_bins, dtype=np.int64)
    nitems = np.zeros(n_bins, dtype=np.int64)
    for c in orderd:
        cand = np.where(nitems < G)[0]
        b = int(cand[np.argmin(sums[cand])])
        bins[b].append(int(c))
        sums[b] += counts[c]
        nitems[b] += 1
    for _ in range(300000):
        dev = sums - target
        over = np.where(dev > 0)[0]
        under = np.where(dev < 0)[0]
        if len(over) == 0 or len(under) == 0:
            break
        A = int(rng.choice(over))
        Bb = int(rng.choice(under))
        ca, cb = bins[A], bins[Bb]
        diff = counts[ca][:, None] - counts[cb][None, :]
        tot = np.abs(dev[A] - diff) + np.abs(dev[Bb] + diff)
        i, j = np.unravel_index(int(np.argmin(tot)), tot.shape)
        if tot[i, j] < abs(dev[A]) + abs(dev[Bb]):
            a, b2 = ca[i], cb[j]
            ca.remove(a), cb.remove(b2)
            ca.append(b2), cb.append(a)
            d = counts[a] - counts[b2]
            sums[A] -= d
            sums[Bb] += d
    bin_of = np.zeros(N_CLUSTERS, dtype=np.int64)
    slot_of = np.zeros(N_CLUSTERS, dtype=np.int64)
    for b, cl in enumerate(bins):
        bin_of[cl] = b
        slot_of[cl] = np.arange(len(cl))
    return bin_of, slot_of, int(sums.max())


def _prepare(output: np.ndarray, mapping: np.ndarray):
    """Host prep: returns (nc, in_maps, cpg, unperm)."""
    t0 = time.time()
    assert output.shape == (32, 8, 512, 512) and output.dtype == np.float32
    mapping = np.asarray(mapping).astype(np.int64).ravel()
    assert mapping.shape == (N,)

    data2d = output.reshape(B, N)
    counts = np.bincount(mapping, minlength=N_CLUSTERS).astype(np.int64)
    recip = (1.0 / np.maximum(counts, 1)).astype(np.float32)

    order = np.argsort(mapping, kind="stable")
    cum = np.zeros(N_CLUSTERS + 1, dtype=np.int64)
    np.cumsum(counts, out=cum[1:])

    n_groups = N_CLUSTERS // G
    # Bin-pack clusters into groups to minimize padding; fall back to
    # consecutive grouping if the packer leaves an oversized bin.
    bin_of, slot_of, maxsum = _solve_bins(counts)
    naive_max = int(np.add.reduceat(counts, np.arange(0, N_CLUSTERS, G)).max())
    if maxsum > naive_max:
        bin_of = np.arange(N_CLUSTERS) // G
        slot_of = np.arange(N_CLUSTERS) % G
        maxsum = naive_max
    cpg = max(1, int(np.ceil(maxsum / 128)))
    L = 128 * cpg

    # clusters in destination order (bin-major, slot order)
    dest_order = np.lexsort((slot_of, bin_of))
    glen = np.zeros(n_groups, dtype=np.int64)
    np.add.at(glen, bin_of, counts)
    rows_sorted = np.concatenate(
        [order[cum[c]:cum[c + 1]] for c in dest_order])
    gstart = np.zeros(n_groups + 1, dtype=np.int64)
    np.cumsum(glen, out=gstart[1:])

    # Padded row-id table [n_groups, L]; -1 = padding.
    pad_rows = np.full((n_groups, L), -1, dtype=np.int64)
    col = np.arange(L)
    valid = col[None, :] < glen[:, None]
    flat_src = np.zeros((n_groups, L), dtype=np.int64)
    flat_src[valid] = rows_sorted[
        (gstart[:-1][:, None] + np.minimum(col[None, :], glen[:, None] - 1))[valid]
    ]
    pad_rows[valid] = flat_src[valid]
    pad_rows = pad_rows.reshape(-1)        # [n_groups * L]
    vmask = pad_rows >= 0

    # Gather data rows (transposed): x_all[r] = data2d[:, pad_rows[r]]
    dataT = np.ascontiguousarray(data2d.T)          # [N, B]
    x_all = np.zeros((n_groups * L, B), dtype=np.float32)
    x_all[vmask] = dataT[pad_rows[vmask]]
    # pack partition-major: [g, t, p, b] -> [g, p, t*B + b]
    x_all = np.ascontiguousarray(
        x_all.reshape(n_groups, cpg, 128, B).transpose(0, 2, 1, 3)
    ).reshape(n_groups, 128, cpg * B)

    # Compact one-hot: per-row within-group column id and value 1/count.
    cid_all = np.zeros(n_groups * L, dtype=np.float32)
    val_all = np.zeros(n_groups * L, dtype=np.float32)
    clus = mapping[pad_rows[vmask]]
    cid_all[vmask] = slot_of[clus].astype(np.float32)
    val_all[vmask] = recip[clus]
    # where cluster c ended up in the concatenated [4096, B] device output
    unperm = bin_of * G + slot_of
    # pack [rows] -> [core][p][chunk]
    nchunks = GROUPS_PER_CORE * cpg

    def pack(a):
        return np.ascontiguousarray(
            a.reshape(NCORES, nchunks, 128).transpose(0, 2, 1))

    cid_all = pack(cid_all)
    val_all = pack(val_all)
    iota_np = np.broadcast_to(np.arange(G, dtype=np.float32), (128, G)).copy()

    t1 = time.time()
    nc = _build_program(cpg)

    in_maps = []
    for k in range(NCORES):
        in_maps.append({
            "x": x_all[k * GROUPS_PER_CORE:(k + 1) * GROUPS_PER_CORE],
            "cid": cid_all[k],
            "val": val_all[k],
            "iota": iota_np,
        })
    print(f"[kernel] host prep {t1 - t0:.2f}s  build+compile "
          f"{time.time() - t1:.2f}s  (cpg={cpg})", file=sys.stderr, flush=True)
    return nc, in_maps, cpg, unperm


def kernel(output: np.ndarray, mapping: np.ndarray) -> np.ndarray:
    nc, in_maps, _, unperm = _prepare(output, mapping)
    t2 = time.time()
    res = run_bass_kernel_spmd(nc, in_maps, list(range(NCORES)))
    t3 = time.time()
    full = np.concatenate([res.results[k]["out"] for k in range(NCORES)],
                          axis=0)                   # [4096, 256] device order
    full = full[unperm]                             # -> cluster order
    out = np.ascontiguousarray(full.T).reshape(32, 8, N_CLUSTERS)
    print(f"[kernel] run {t3 - t2:.2f}s", file=sys.stderr, flush=True)
    return out


# revision 23
# speedup vs baseline: 1.1722x; 1.0192x over previous
"""Segment-mean (MeanToERA5) Trainium2 kernel.

Computes per-cluster means of a [32, 8, 512, 512] fp32 tensor over the
flattened 512x512 spatial axis, for 4096 clusters given by `mapping`
([262144] int), matching jax.ops.segment_sum(flat.T, mapping)/counts.

Strategy (8 NeuronCores, SPMD):
  - Host: stable-argsort `mapping`; group the 4096 clusters into groups of
    G=32 consecutive clusters; lay out the data cluster-sorted and
    transposed as rows of [256 batch] fp32, padded per-group to a uniform
    row count 128*cpg so the program structure is identical on every
    core. Each core owns 512 clusters = 16 groups. Inputs are packed
    partition-major on the host so all DMAs are fully contiguous.
  - Device: build the per-chunk [128, 32] one-hot weights on DVE from
    compact (column-id, 1/count) vectors; per 128-row chunk one fp32
    matmul: stationary = one-hot, moving = data chunk [128, 256]. PSUM
    accumulates [512 clusters, 256 batch] c-major in 4 [128, 256] tiles;
    copy + DMA out at the end.
  - Host: assemble [4096, 256], transpose to [256, 4096] (the unshard).
"""

import sys
import time

if "/opt/trn_rl_repo" not in sys.path:
    sys.path.insert(0, "/opt/trn_rl_repo")

import numpy as np

import concourse.bacc as bacc
import concourse.tile as tile
from concourse import mybir
from concourse.bass_utils import run_bass_kernel_spmd

N_CLUSTERS = 4096
N = 512 * 512
B = 256
NCORES = 8
G = 32                      # clusters per group (= one-hot width)
GROUPS_PER_CORE = (N_CLUSTERS // NCORES) // G   # 16
CLUSTERS_PER_CORE = N_CLUSTERS // NCORES        # 512
NQ = CLUSTERS_PER_CORE // 128                   # psum tiles (4)

_program_cache = {}
LAST_EXEC_NS = None


def _build_program(cpg: int, loop: int = 1):
    """Build the SPMD bass program for `cpg` 128-row chunks per group.

    loop > 1 repeats the whole pipeline on-device (for benchmarking: one
    dispatch, `loop` executions)."""
    key = (cpg, loop)
    if key in _program_cache:
        return _program_cache[key]

    nchunks = GROUPS_PER_CORE * cpg    # chunks per core
    gpq = 128 // G                     # groups per psum tile (4)

    nc = bacc.Bacc("TRN2", target_bir_lowering=False, debug=False,
                   num_devices=NCORES)
    # x packed as [groups, 128 partitions, cpg*B] (host pre-permuted)
    x = nc.dram_tensor("x", [GROUPS_PER_CORE, 128, cpg * B],
                       mybir.dt.float32, kind="ExternalInput")
    # per-row one-hot column id and value, packed [128, nchunks]
    cid = nc.dram_tensor("cid", [128, nchunks], mybir.dt.float32,
                         kind="ExternalInput")
    val = nc.dram_tensor("val", [128, nchunks], mybir.dt.float32,
                         kind="ExternalInput")
    iota = nc.dram_tensor("iota", [128, G], mybir.dt.float32,
                          kind="ExternalInput")
    # output c-major: [512 clusters, 256 batch]
    out = nc.dram_tensor("out", [CLUSTERS_PER_CORE, B], mybir.dt.float32,
                         kind="ExternalOutput")

    xv, outv = x.ap(), out.ap()

    with tile.TileContext(nc) as tc:
        with (
            tc.tile_pool(name="xp", bufs=6) as xp,
            tc.tile_pool(name="ohp", bufs=1) as ohp,
            tc.tile_pool(name="ps", bufs=1, space="PSUM") as ps,
            tc.tile_pool(name="res", bufs=2) as resp,
        ):
            def body(_i=None):
                cidt = ohp.tile([128, nchunks], mybir.dt.float32,
                                name="cidt", tag="cidt")
                nc.sync.dma_start(cidt[:], cid.ap())
                valt = ohp.tile([128, nchunks], mybir.dt.float32,
                                name="valt", tag="valt")
                nc.sync.dma_start(valt[:], val.ap())
                iot = ohp.tile([128, G], mybir.dt.float32,
                               name="iot", tag="iot")
                nc.sync.dma_start(iot[:], iota.ap())
                # expand to one-hot weights [128, nchunks, G] (per group,
                # so matmuls can start as soon as the first slice is ready)
                ohx = ohp.tile([128, nchunks, G], mybir.dt.float32,
                               name="ohx", tag="ohx")
                for g in range(GROUPS_PER_CORE):
                    s = slice(g * cpg, (g + 1) * cpg)
                    nc.vector.tensor_tensor(
                        out=ohx[:, s, :],
                        in0=cidt[:, s].unsqueeze(2)
                            .broadcast_to([128, cpg, G]),
                        in1=iot[:].unsqueeze(1).broadcast_to([128, cpg, G]),
                        op=mybir.AluOpType.is_equal,
                    )
                    nc.vector.tensor_tensor(
                        out=ohx[:, s, :],
                        in0=ohx[:, s, :],
                        in1=valt[:, s].unsqueeze(2)
                            .broadcast_to([128, cpg, G]),
                        op=mybir.AluOpType.mult,
                    )
                psum = [
                    ps.tile([128, B], mybir.dt.float32,
                            name=f"psum{q}", tag=f"psum{q}")
                    for q in range(NQ)
                ]
                for g in range(GROUPS_PER_CORE):
                    q, gq = divmod(g, gpq)
                    po = gq * G        # partition offset within psum tile
                    xt = xp.tile([128, cpg * B], mybir.dt.float32, tag="xt")
                    nc.sync.dma_start(xt[:], xv[g])
                    for t in range(cpg):
                        j = g * cpg + t
                        nc.tensor.matmul(
                            out=psum[q][po:po + G, :],
                            lhsT=ohx[:, j, :],
                            rhs=xt[:, t * B:(t + 1) * B],
                            start=(t == 0),
                            stop=(t == cpg - 1),
                            tile_position=(0, po),
                        )
                for q in range(NQ):
                    res = resp.tile([128, B], mybir.dt.float32,
                                    name=f"res{q}", tag="res")
                    nc.vector.tensor_copy(res[:], psum[q][:])
                    nc.sync.dma_start(outv[q * 128:(q + 1) * 128, :], res[:])

            if loop == 1:
                body()
            else:
                with tc.For_i(0, loop, 1) as i:
                    body(i)

    nc.compile()
    _program_cache[key] = nc
    return nc


def _solve_bins(counts: np.ndarray):
    """Partition the 4096 clusters into 128 bins of exactly 32 clusters,
    equalizing bin row-sums (ideally all == 2048 -> zero padding). Returns
    (bin_of, slot_of) int arrays."""
    n_bins = N_CLUSTERS // G
    target = int(counts.sum()) // n_bins
    rng = np.random.default_rng(0)
    orderd = np.argsort(-counts)
    bins = [[] for _ in range(n_bins)]
    sums = np.zeros(n_bins, dtype=np.int64)
    nitems = np.zeros(n_bins, dtype=np.int64)
    for c in orderd:
        cand = np.where(nitems < G)[0]
        b = int(cand[np.argmin(sums[cand])])
        bins[b].append(int(c))
        sums[b] += counts[c]
        nitems[b] += 1
    for _ in range(300000):
        dev = sums - target
        over = np.where(dev > 0)[0]
        under = np.where(dev < 0)[0]
        if len(over) == 0 or len(under) == 0:
            break
        A = int(rng.choice(over))
        Bb = int(rng.choice(under))
        ca, cb = bins[A], bins[Bb]
        diff = counts[ca][:, None] - counts[cb][None, :]
        tot = np.abs(dev[A] - diff) + np.abs(dev[Bb] + diff)
        i, j = np.unravel_index(int(np.argmin(tot)), tot.shape)
        if tot[i, j] < abs(dev[A]) + abs(dev[Bb]):
            a, b2 = ca[i], cb[j]
            ca.remove(a), cb.remove(b2)
            ca.append(b2), cb.append(a)
            d = counts[a] - counts[b2]
            sums[A] -= d
            sums[Bb] += d
    bin_of = np.zeros(N_CLUSTERS, dtype=np.int64)
    slot_of = np.zeros(N_CLUSTERS, dtype=np.int64)
    for b, cl in enumerate(bins):
        bin_of[cl] = b
        slot_of[cl] = np.arange(len(cl))
    return bin_of, slot_of, int(sums.max())


def _prepare(output: np.ndarray, mapping: np.ndarray):
    """Host prep: returns (nc, in_maps, cpg, unperm)."""
    t0 = time.time()
    assert output.shape == (32, 8, 512, 512) and output.dtype == np.float32
    mapping = np.asarray(mapping).astype(np.int64).ravel()
    assert mapping.shape == (N,)

    data2d = output.reshape(B, N)
    counts = np.bincount(mapping, minlength=N_CLUSTERS).astype(np.int64)
    recip = (1.0 / np.maximum(counts, 1)).astype(np.float32)

    order = np.argsort(mapping, kind="stable")
    cum = np.zeros(N_CLUSTERS + 1, dtype=np.int64)
    np.cumsum(counts, out=cum[1:])

    n_groups = N_CLUSTERS // G
    # Bin-pack clusters into groups to minimize padding; fall back to
    # consecutive grouping if the packer leaves an oversized bin.
    bin_of, slot_of, maxsum = _solve_bins(counts)
    naive_max = int(np.add.reduceat(counts, np.arange(0, N_CLUSTERS, G)).max())
    if maxsum > naive_max:
        bin_of = np.arange(N_CLUSTERS) // G
        slot_of = np.arange(N_CLUSTERS) % G
        maxsum = naive_max
    cpg = max(1, int(np.ceil(maxsum / 128)))
    L = 128 * cpg

    # clusters in destination order (bin-major, slot order)
    dest_order = np.lexsort((slot_of, bin_of))
    glen = np.zeros(n_groups, dtype=np.int64)
    np.add.at(glen, bin_of, counts)
    rows_sorted = np.concatenate(
        [order[cum[c]:cum[c + 1]] for c in dest_order])
    gstart = np.zeros(n_groups + 1, dtype=np.int64)
    np.cumsum(glen, out=gstart[1:])

    # Padded row-id table [n_groups, L]; -1 = padding.
    pad_rows = np.full((n_groups, L), -1, dtype=np.int64)
    col = np.arange(L)
    valid = col[None, :] < glen[:, None]
    flat_src = np.zeros((n_groups, L), dtype=np.int64)
    flat_src[valid] = rows_sorted[
        (gstart[:-1][:, None] + np.minimum(col[None, :], glen[:, None] - 1))[valid]
    ]
    pad_rows[valid] = flat_src[valid]
    pad_rows = pad_rows.reshape(-1)        # [n_groups * L]
    vmask = pad_rows >= 0

    # Gather data rows (transposed): x_all[r] = data2d[:, pad_rows[r]]
    dataT = np.ascontiguousarray(data2d.T)          # [N, B]
    x_all = np.zeros((n_groups * L, B), dtype=np.float32)
    x_all[vmask] = dataT[pad_rows[vmask]]
    # pack partition-major: [g, t, p, b] -> [g, p, t*B + b]
    x_all = np.ascontiguousarray(
        x_all.reshape(n_groups, cpg, 128, B).transpose(0, 2, 1, 3)
    ).reshape(n_groups, 128, cpg * B)

    # Compact one-hot: per-row within-group column id and value 1/count.
    cid_all = np.zeros(n_groups * L, dtype=np.float32)
    val_all = np.zeros(n_groups * L, dtype=np.float32)
    clus = mapping[pad_rows[vmask]]
    cid_all[vmask] = slot_of[clus].astype(np.float32)
    val_all[vmask] = recip[clus]
    # where cluster c ended up in the concatenated [4096, B] device output
    unperm = bin_of * G + slot_of
    # pack [rows] -> [core][p][chunk]
    nchunks = GROUPS_PER_CORE * cpg

    def pack(a):
        return np.ascontiguousarray(
            a.reshape(NCORES, nchunks, 128).transpose(0, 2, 1))

    cid_all = pack(cid_all)
    val_all = pack(val_all)
    iota_np = np.broadcast_to(np.arange(G, dtype=np.float32), (128, G)).copy()

    t1 = time.time()
    nc = _build_program(cpg)

    in_maps = []
    for k in range(NCORES):
        in_maps.append({
            "x": x_all[k * GROUPS_PER_CORE:(k + 1) * GROUPS_PER_CORE],
            "cid": cid_all[k],
            "val": val_all[k],
            "iota": iota_np,
        })
    print(f"[kernel] host prep {t1 - t0:.2f}s  build+compile "
          f"{time.time() - t1:.2f}s  (cpg={cpg})", file=sys.stderr, flush=True)
    return nc, in_maps, cpg, unperm


def kernel(output: np.ndarray, mapping: np.ndarray) -> np.ndarray:
    nc, in_maps, _, unperm = _prepare(output, mapping)
    t2 = time.time()
    res = run_bass_kernel_spmd(nc, in_maps, list(range(NCORES)))
    t3 = time.time()
    full = np.concatenate([res.results[k]["out"] for k in range(NCORES)],
                          axis=0)                   # [4096, 256] device order
    full = full[unperm]                             # -> cluster order
    out = np.ascontiguousarray(full.T).reshape(32, 8, N_CLUSTERS)
    print(f"[kernel] run {t3 - t2:.2f}s", file=sys.stderr, flush=True)
    return out
